# revision 30
# baseline (speedup 1.0000x reference)
"""Trainium2 Bass kernel for a top-2 MoE layer (T=2048, H=2048, I=1408, E=8).

Strategy: expert-parallel over 8 NeuronCores. The host dispatches tokens:
for each expert e it gathers the tokens routed to e (~480 of 2048, padded
to a shared capacity C sized to the busiest expert), so each core runs a
dense [C,H] FFN for its expert. The host then combines per-expert outputs
with the routing weights.

Device kernel (per core), transposed layout (no on-device transposes):
  warmup : 10 matmuls on a zeroed SBUF tile run during the initial DMA
           wait so the PE HAM clock-gate reaches 8/8 (2.4 GHz) before the
           first real matmul; the real stream then runs fully warm.
  stage 1: guT[2816, C] = w13 @ xT   (per row-block m: g chain then u
           chain over 16 K-tiles; block 0 is split into k-halves so its
           chains track the ~358GB/s HBM supply instead of outrunning it)
  stage 2: actT[1408, C] = silu(gT) * uT   (ScalarE Silu + VectorE mul)
  stage 3: yT[2048, C] = w2 @ actT, PSUM->SBUF copy on VectorE as fp16
           (halves the output DMA bytes; adds ~1e-4 rel-err); stores go
           out on the scalar HWDGE ring, off the weight-loading ring.

Matmuls in fp16 (full PE rate, half the DMA bytes of fp32; fp32 PSUM
accumulation keeps rel-err ~6e-4). DMA plan: first row-block's weights as
quarter transfers interleaved with five x chunks ordered by first use
(the first chunk is a single k-tile so the first matmul's deps are tiny);
later w13 row-blocks as half-row transfers paced by a 6-slot pool; w2 as
row-block pairs prefetched inside stage 3 so they never steal bandwidth
from the stage-1 stream. All DRAM sources are contiguous ranges (strided
column-slice sources cost ~2x descriptor work and drop supply to
~322GB/s). The ramp runs m0 k0-7, then all of m1, then m0 k8-15 so the
deferred half never waits on DMA completion semaphores, with filler
warmup matmuls inside the paced section to keep the HAM clock-gate at
8/8. Measured ~130.9-132.8us (median 131.8) vs the 135.4us baseline;
the matmul stream runs at the 2.4GHz issue-rate floor (528 x 211ns) and
the remaining gap is fixed NEFF preamble/epilogue and DMA
completion-semaphore latency.
"""

import sys

if "/opt/trn_rl_repo" not in sys.path:
    sys.path.insert(0, "/opt/trn_rl_repo")

import os
import numpy as np
from contextlib import ExitStack

import concourse.bass as bass
import concourse.tile as tile
from concourse import bacc, mybir

T, H, I, E, K = 2048, 2048, 1408, 8, 2
CMAX = 512                   # max token capacity per expert per pass (PSUM bank)
HT = H // 128                # 16 K-tiles over H
IT = I // 128                # 11 K-tiles over I
BT = 2 * I // 128            # 22 row-blocks of guT
XSPLIT = (1, 3, 4, 4, 4)     # x chunk sizes in k-tiles (first-use ordered)
NWARM = 9                    # PE warmup matmuls (>=3.4us of PE activity so
                             # the HAM window flips before the real stream)

DT = mybir.dt.float16
NP_DT = np.float16
F32 = mybir.dt.float32

_cache: dict = {}


def _build_nc(C):
    """Build + compile the per-core FFN program (same program on all cores)."""
    nc = bacc.Bacc("TRN2", target_bir_lowering=False, debug=False, num_devices=E)
    GW = HT * 128  # columns of one g (or u) row-block
    # every DMA source below is a fully contiguous DRAM range: strided
    # column-slice sources cost extra descriptors (issue 0.9-1.4us instead
    # of ~0.6us) and drop sustained supply from ~358 to ~322GB/s
    # x chunk q: k-tiles sum(XSPLIT[:q]).. side by side, [128, nk*C]
    x_ds = [
        nc.dram_tensor(f"x{q}", [128, XSPLIT[q] * C], DT, kind="ExternalInput")
        for q in range(len(XSPLIT))
    ]
    # row-block 0 of w13 as four quarters: g k0-7, u k0-7, g k8-15, u k8-15
    w0_d = nc.dram_tensor("w0q", [4, 128, GW // 2], DT, kind="ExternalInput")
    # w13 halves: row 2m = g block m, row 2m+1 = u block m (rows 0,1 unused)
    w13_d = nc.dram_tensor("w13h", [2 * IT, 128, GW], DT, kind="ExternalInput")
    # w2 pair p: [block 2p | block 2p+1], each [128, IT*128]
    w2_d = nc.dram_tensor("w2_sb", [HT // 2, 128, 2 * IT * 128], DT, kind="ExternalInput")
    y_d = nc.dram_tensor("y_sb", [HT, 128, C], DT, kind="ExternalOutput")

    AF = mybir.ActivationFunctionType

    with tile.TileContext(nc) as tc, ExitStack() as ctx:
        zp = ctx.enter_context(tc.tile_pool(name="z", bufs=1))
        xp = ctx.enter_context(tc.tile_pool(name="x", bufs=1))
        w0p = ctx.enter_context(tc.tile_pool(name="w0", bufs=1))
        wp = ctx.enter_context(tc.tile_pool(name="w", bufs=6))
        w2p = ctx.enter_context(tc.tile_pool(name="w2", bufs=2))
        ap = ctx.enter_context(tc.tile_pool(name="act", bufs=1))
        sp = ctx.enter_context(tc.tile_pool(name="tmp", bufs=2))
        yp = ctx.enter_context(tc.tile_pool(name="y", bufs=3))
        psg = ctx.enter_context(
            tc.tile_pool(name="psg", bufs=5, space=bass.MemorySpace.PSUM)
        )
        psy = ctx.enter_context(
            tc.tile_pool(name="psy", bufs=3, space=bass.MemorySpace.PSUM)
        )

        # --- PE warmup: no DMA deps, runs during the initial DMA wait -----
        zw = zp.tile([128, 128], DT, tag="zw")
        nc.gpsimd.memset(zw[:], 0.0)
        zx = zp.tile([128, C], DT, tag="zx")
        nc.gpsimd.memset(zx[:], 0.0)
        warm_ps = psg.tile([128, C], F32, tag="ps")
        for i in range(NWARM):
            nc.tensor.matmul(warm_ps[:], zw[:], zx[:], start=True, stop=True)

        # --- DMA issue schedule (ordered by first-use time) ----------------
        # m=0/halves of m=1,2 weights as half transfers interleaved with
        # the x chunks; x chunk 0 is a single k-tile so the first matmul's
        # deps are tiny. All on the sync HWDGE ring, in consumption order.
        w0 = {}
        def _load_w0(which, half):
            # which: 0=g, 1=u; half: 0 = k-tiles 0..7, 1 = k-tiles 8..15
            t = w0p.tile([128, GW // 2], DT, tag=f"w0_{which}_{half}")
            nc.sync.dma_start(t[:], w0_d.ap()[2 * half + which])
            w0[(which, half)] = t

        x_t = []      # (tile, k_start, n_k)
        def _load_x(q):
            k0 = sum(XSPLIT[:q])
            nk = XSPLIT[q]
            xt = xp.tile([128, nk * C], DT, tag=f"x{q}")
            nc.sync.dma_start(xt[:], x_ds[q].ap())
            x_t.append((xt, k0, nk))

        wgu = {}
        def _load_w13_half(m, which):
            # one g (which=0) or u (which=1) row-block of w13, 512KB
            t = wp.tile([128, GW], DT, tag="w13")
            nc.sync.dma_start(t[:], w13_d.ap()[2 * m + which])
            wgu[(m, which)] = t

        # Issue order = consumption order of the restructured ramp below:
        # m0 k0-7 halves, then all of m1, then m0 k8-15, then m2...
        _load_x(0)       # k0: first matmul dep (tiny, lands first)
        _load_w0(0, 0)   # wg k0-7
        _load_x(1)       # k1-3
        _load_x(2)       # k4-7
        _load_w0(1, 0)   # wu k0-7
        _load_w13_half(1, 0)
        _load_x(3)       # k8-11
        _load_x(4)       # k12-15 (before wu1: m1's g-chain consumes it first)
        _load_w13_half(1, 1)
        _load_w0(0, 1)   # wg k8-15
        _load_w0(1, 1)   # wu k8-15
        _load_w13_half(2, 0)
        _load_w13_half(2, 1)

        def xsl(k):
            for xt, k0, nk in x_t:
                if k0 <= k < k0 + nk:
                    return xt[:, (k - k0) * C: (k - k0 + 1) * C]
            raise AssertionError(k)

        # --- stage 1 + 2 ---------------------------------------------------
        # Ramp order: m0 k0-7 (paced by the x chunk supply), then ALL of
        # m1 (its weights land early), then m0 k8-15 (deps long-arrived by
        # now, so no per-chunk completion-semaphore stalls), then m2..m10.
        act_t = [None] * IT

        def _weights(m):
            for which in (0, 1):
                if (m, which) not in wgu:
                    _load_w13_half(m, which)
            g_t = wgu.pop((m, 0))
            u_t = wgu.pop((m, 1))
            return ([g_t[:, k * 128: (k + 1) * 128] for k in range(HT)],
                    [u_t[:, k * 128: (k + 1) * 128] for k in range(HT)])

        def _chains(g_w, u_w, g_ps, u_ps, ks, start, stop):
            for k in ks:
                nc.tensor.matmul(g_ps[:], g_w[k], xsl(k),
                                 start=(start and k == ks[0]),
                                 stop=(stop and k == ks[-1]))
            for k in ks:
                nc.tensor.matmul(u_ps[:], u_w[k], xsl(k),
                                 start=(start and k == ks[0]),
                                 stop=(stop and k == ks[-1]))

        def _finish(m, g_ps, u_ps):
            sg = sp.tile([128, C], F32, tag="sg")
            nc.scalar.activation(sg[:], g_ps[:], AF.Silu)
            at = ap.tile([128, C], DT, tag=f"act{m}")
            nc.vector.tensor_mul(at[:], sg[:], u_ps[:])
            act_t[m] = at

        g_w0 = [w0[(0, k // 8)][:, (k % 8) * 128: (k % 8 + 1) * 128]
                for k in range(HT)]
        u_w0 = [w0[(1, k // 8)][:, (k % 8) * 128: (k % 8 + 1) * 128]
                for k in range(HT)]
        g_ps0 = psg.tile([128, C], F32, tag="ps")
        u_ps0 = psg.tile([128, C], F32, tag="ps")
        # filler warmups inside the DMA-paced ramp keep PE activity dense
        # through the x-chunk waits, so the HAM clock-gate stays at 8/8
        for k in range(HT // 2):
            nc.tensor.matmul(g_ps0[:], g_w0[k], xsl(k),
                             start=(k == 0), stop=False)
            if k in (0, 3, 5):
                nc.tensor.matmul(warm_ps[:], zw[:], zx[:], start=True, stop=True)
                nc.tensor.matmul(warm_ps[:], zw[:], zx[:], start=True, stop=True)
        for k in range(HT // 2):
            nc.tensor.matmul(u_ps0[:], u_w0[k], xsl(k),
                             start=(k == 0), stop=False)

        g_w1, u_w1 = _weights(1)
        g_ps1 = psg.tile([128, C], F32, tag="ps")
        u_ps1 = psg.tile([128, C], F32, tag="ps")
        _chains(g_w1, u_w1, g_ps1, u_ps1, list(range(HT)),
                start=True, stop=True)
        _finish(1, g_ps1, u_ps1)

        _chains(g_w0, u_w0, g_ps0, u_ps0, list(range(HT // 2, HT)),
                start=False, stop=True)
        _finish(0, g_ps0, u_ps0)

        for m in range(2, IT):
            g_w, u_w = _weights(m)
            g_ps = psg.tile([128, C], F32, tag="ps")
            u_ps = psg.tile([128, C], F32, tag="ps")
            _chains(g_w, u_w, g_ps, u_ps, list(range(HT)),
                    start=True, stop=True)
            _finish(m, g_ps, u_ps)

        # --- stage 3 -------------------------------------------------------
        # w2 pair p is DMA'd two blocks ahead of first use (pairs 0,1 queue
        # behind the last w13 transfers; later pairs issue inside the loop).
        w2t = {}
        def _load_w2(p):
            t = w2p.tile([128, 2 * IT * 128], DT, tag=f"w2_{p % 2}")
            nc.sync.dma_start(t[:], w2_d.ap()[p])
            w2t[p] = t

        _load_w2(0)
        _load_w2(1)
        for m in range(HT):
            p = m // 2
            if m % 2 == 0 and p + 2 <= HT // 2 - 1:
                _load_w2(p + 2)
            wt = w2t[p]
            base = (m % 2) * IT * 128
            y_ps = psy.tile([128, C], F32, tag="y")
            for k in range(IT):
                nc.tensor.matmul(
                    y_ps[:], wt[:, base + k * 128: base + (k + 1) * 128],
                    act_t[k][:], start=(k == 0), stop=(k == IT - 1),
                )
            if m % 2 == 1:
                del w2t[p]
            y_sb = yp.tile([128, C], DT, tag="yout")
            nc.vector.tensor_copy(y_sb[:], y_ps[:])
            # scalar HWDGE ring: keeps y stores off the sync ring's queue
            nc.scalar.dma_start(y_d.ap()[m], y_sb[:])

    nc.compile()
    return nc


def _get_nc(C):
    if C not in _cache:
        _cache[C] = _build_nc(C)
    return _cache[C]


def _prep_weights(w13, w2):
    """Pre-tile weights into the SBUF layouts the kernel DMAs verbatim."""
    GW = HT * 128
    wb = (
        w13.reshape(E, BT, 128, HT, 128)
        .transpose(0, 1, 4, 3, 2)
        .astype(NP_DT)
        .reshape(E, BT, 128, GW)
    )
    # halves interleaved: row 2m = g block m, row 2m+1 = u block m
    w13h = np.stack([wb[:, :IT], wb[:, IT:]], axis=2).reshape(E, 2 * IT, 128, GW)
    # block-0 quarters in first-use order: g k0-7, u k0-7, g k8-15, u k8-15
    w0q = np.stack(
        [wb[:, 0, :, : GW // 2], wb[:, IT, :, : GW // 2],
         wb[:, 0, :, GW // 2:], wb[:, IT, :, GW // 2:]],
        axis=1,
    )
    w2b = (
        w2.reshape(E, HT, 128, IT, 128)
        .transpose(0, 1, 4, 3, 2)
        .astype(NP_DT)
        .reshape(E, HT, 128, IT * 128)
    )
    w2_sb = np.concatenate([w2b[:, 0::2], w2b[:, 1::2]], axis=3)
    return (np.ascontiguousarray(w0q), np.ascontiguousarray(w13h),
            np.ascontiguousarray(w2_sb))


def _prep_x(xe, C):
    """[C, H] fp32 -> per-chunk arrays [128, nk*C] fp16, k-tiles side by side."""
    xt = xe.T.reshape(HT, 128, C).transpose(1, 0, 2)
    flat = np.ascontiguousarray(xt).reshape(128, HT * C).astype(NP_DT)
    out = {}
    k0 = 0
    for q, nk in enumerate(XSPLIT):
        out[f"x{q}"] = np.ascontiguousarray(flat[:, k0 * C: (k0 + nk) * C])
        k0 += nk
    return out


def kernel(
    hidden_states,
    topk_weights,
    topk_ids,
    w13,
    w2,
    num_global_tokens=None,
    max_num_tokens_per_gpu=None,
):
    from concourse.bass_utils import run_bass_kernel_spmd

    hs = np.asarray(hidden_states, dtype=np.float32)
    tw = np.asarray(topk_weights, dtype=np.float32)
    ti = np.asarray(topk_ids)
    w13 = np.asarray(w13, dtype=np.float32)
    w2 = np.asarray(w2, dtype=np.float32)

    assert hs.shape == (T, H), hs.shape
    assert w13.shape == (E, 2 * I, H), w13.shape
    assert w2.shape == (E, H, I), w2.shape

    # per-(token, expert) combine weights: sum of topk weights routed to e
    # (out-of-range ids contribute nothing, matching jax.nn.one_hot)
    comb = np.zeros((T, E), dtype=np.float32)
    for k in range(ti.shape[1]):
        col = ti[:, k]
        ok = (col >= 0) & (col < E)
        np.add.at(comb, (np.arange(T)[ok], col[ok]), tw[ok, k])

    idxs = [np.nonzero(comb[:, e])[0] for e in range(E)]
    need = max(len(ix) for ix in idxs)
    # token capacity: matmul N dim, multiple of 4 (x chunk layout)
    C = min(CMAX, max(64, -(-need // 4) * 4))
    nchunks = max(1, -(-need // C))

    w0q, w13h, w2_sb = _prep_weights(w13, w2)
    nc = _get_nc(C)

    trace = bool(os.environ.get("KERNEL_PROFILE"))
    out = np.zeros((T, H), dtype=np.float32)
    for chunk in range(nchunks):
        in_maps = []
        sels = []
        for e in range(E):
            sel = idxs[e][chunk * C: (chunk + 1) * C]
            xe = np.zeros((C, H), dtype=np.float32)
            xe[: len(sel)] = hs[sel]
            im = {"w0q": w0q[e], "w13h": w13h[e], "w2_sb": w2_sb[e]}
            im.update(_prep_x(xe, C))
            in_maps.append(im)
            sels.append(sel)
        if trace:
            try:
                res = run_bass_kernel_spmd(nc, in_maps, list(range(E)), trace=True)
                if res.exec_time_ns is not None:
                    print(f"HW exec time: {res.exec_time_ns} ns")
            except Exception:
                res = run_bass_kernel_spmd(nc, in_maps, list(range(E)))
        else:
            res = run_bass_kernel_spmd(nc, in_maps, list(range(E)))
        for e in range(E):
            sel = sels[e]
            if len(sel) == 0:
                continue
            y_sb = np.asarray(res.results[e]["y_sb"], dtype=np.float32)
            ye = y_sb.reshape(H, C).T  # [C, H]
            out[sel] += comb[sel, e][:, None] * ye[: len(sel)]
    return out


# revision 31
# speedup vs baseline: 1.0063x; 1.0063x over previous
"""Trainium2 Bass kernel for a top-2 MoE layer (T=2048, H=2048, I=1408, E=8).

Strategy: expert-parallel over 8 NeuronCores. The host dispatches tokens:
for each expert e it gathers the tokens routed to e (~480 of 2048, padded
to a shared capacity C sized to the busiest expert), so each core runs a
dense [C,H] FFN for its expert. The host then combines per-expert outputs
with the routing weights.

Device kernel (per core), transposed layout (no on-device transposes):
  warmup : 9 matmuls on a zeroed SBUF tile run during the initial DMA
           wait (plus 6 fillers inside the DMA-paced ramp) so the PE HAM
           clock-gate reaches and stays at 8/8 (2.4 GHz); the real
           stream then runs fully warm.
  stage 1: guT[2816, C] = w13 @ xT   (per row-block m: g chain then u
           chain over 16 K-tiles; block 0 is split into k-halves so its
           chains track the ~358GB/s HBM supply instead of outrunning it)
  stage 2: actT[1408, C] = silu(gT) * uT   (ScalarE Silu + VectorE mul)
  stage 3: yT[2048, C] = w2 @ actT, PSUM->SBUF copy on VectorE as fp16
           (halves the output DMA bytes; adds ~1e-4 rel-err); stores go
           out on the scalar HWDGE ring, off the weight-loading ring.

Matmuls in fp16 (full PE rate, half the DMA bytes of fp32; fp32 PSUM
accumulation keeps rel-err ~6e-4). DMA plan: first row-block's weights as
quarter transfers interleaved with five x chunks ordered by first use
(the first chunk is a single k-tile so the first matmul's deps are tiny);
later w13 row-blocks as half-row transfers paced by a 6-slot pool; w2 as
row-block pairs prefetched inside stage 3 so they never steal bandwidth
from the stage-1 stream. All DRAM sources are contiguous ranges (strided
column-slice sources cost ~2x descriptor work and drop supply to
~322GB/s). The ramp runs m0 k0-7, then all of m1, then m0 k8-15 so the
deferred half never waits on DMA completion semaphores, with filler
warmup matmuls inside the paced section to keep the HAM clock-gate at
8/8. Measured ~130.9-132.8us (median 131.8) vs the 135.4us baseline;
the matmul stream runs at the 2.4GHz issue-rate floor (528 x 211ns) and
the remaining gap is fixed NEFF preamble/epilogue and DMA
completion-semaphore latency.
"""

import sys

if "/opt/trn_rl_repo" not in sys.path:
    sys.path.insert(0, "/opt/trn_rl_repo")

import os
import numpy as np
from contextlib import ExitStack

import concourse.bass as bass
import concourse.tile as tile
from concourse import bacc, mybir

T, H, I, E, K = 2048, 2048, 1408, 8, 2
CMAX = 512                   # max token capacity per expert per pass (PSUM bank)
HT = H // 128                # 16 K-tiles over H
IT = I // 128                # 11 K-tiles over I
BT = 2 * I // 128            # 22 row-blocks of guT
XSPLIT = (1, 3, 4, 4, 4)     # x chunk sizes in k-tiles (first-use ordered)
NWARM = 9                    # PE warmup matmuls (>=3.4us of PE activity so
                             # the HAM window flips before the real stream)

DT = mybir.dt.float16
NP_DT = np.float16
F32 = mybir.dt.float32

_cache: dict = {}


def _build_nc(C):
    """Build + compile the per-core FFN program (same program on all cores)."""
    nc = bacc.Bacc("TRN2", target_bir_lowering=False, debug=False, num_devices=E)
    GW = HT * 128  # columns of one g (or u) row-block
    # every DMA source below is a fully contiguous DRAM range: strided
    # column-slice sources cost extra descriptors (issue 0.9-1.4us instead
    # of ~0.6us) and drop sustained supply from ~358 to ~322GB/s
    # x chunk q: k-tiles sum(XSPLIT[:q]).. side by side, [128, nk*C]
    x_ds = [
        nc.dram_tensor(f"x{q}", [128, XSPLIT[q] * C], DT, kind="ExternalInput")
        for q in range(len(XSPLIT))
    ]
    # row-block 0 of w13 as four quarters: g k0-7, u k0-7, g k8-15, u k8-15
    w0_d = nc.dram_tensor("w0q", [4, 128, GW // 2], DT, kind="ExternalInput")
    # w13 halves: row 2m = g block m, row 2m+1 = u block m (rows 0,1 unused)
    w13_d = nc.dram_tensor("w13h", [2 * IT, 128, GW], DT, kind="ExternalInput")
    # w2 pair p: [block 2p | block 2p+1], each [128, IT*128]
    w2_d = nc.dram_tensor("w2_sb", [HT // 2, 128, 2 * IT * 128], DT, kind="ExternalInput")
    y_d = nc.dram_tensor("y_sb", [HT, 128, C], DT, kind="ExternalOutput")

    AF = mybir.ActivationFunctionType

    with tile.TileContext(nc) as tc, ExitStack() as ctx:
        zp = ctx.enter_context(tc.tile_pool(name="z", bufs=1))
        xp = ctx.enter_context(tc.tile_pool(name="x", bufs=1))
        w0p = ctx.enter_context(tc.tile_pool(name="w0", bufs=1))
        wp = ctx.enter_context(tc.tile_pool(name="w", bufs=6))
        w2p = ctx.enter_context(tc.tile_pool(name="w2", bufs=2))
        ap = ctx.enter_context(tc.tile_pool(name="act", bufs=1))
        sp = ctx.enter_context(tc.tile_pool(name="tmp", bufs=2))
        yp = ctx.enter_context(tc.tile_pool(name="y", bufs=3))
        psg = ctx.enter_context(
            tc.tile_pool(name="psg", bufs=5, space=bass.MemorySpace.PSUM)
        )
        psy = ctx.enter_context(
            tc.tile_pool(name="psy", bufs=3, space=bass.MemorySpace.PSUM)
        )

        # --- PE warmup: no DMA deps, runs during the initial DMA wait -----
        zw = zp.tile([128, 128], DT, tag="zw")
        nc.gpsimd.memset(zw[:], 0.0)
        zx = zp.tile([128, C], DT, tag="zx")
        nc.gpsimd.memset(zx[:], 0.0)
        warm_ps = psg.tile([128, C], F32, tag="ps")
        for i in range(NWARM):
            nc.tensor.matmul(warm_ps[:], zw[:], zx[:], start=True, stop=True)

        # --- DMA issue schedule (ordered by first-use time) ----------------
        # m=0/halves of m=1,2 weights as half transfers interleaved with
        # the x chunks; x chunk 0 is a single k-tile so the first matmul's
        # deps are tiny. All on the sync HWDGE ring, in consumption order.
        w0 = {}
        def _load_w0(which, half):
            # which: 0=g, 1=u; half: 0 = k-tiles 0..7, 1 = k-tiles 8..15
            t = w0p.tile([128, GW // 2], DT, tag=f"w0_{which}_{half}")
            nc.sync.dma_start(t[:], w0_d.ap()[2 * half + which])
            w0[(which, half)] = t

        x_t = []      # (tile, k_start, n_k)
        def _load_x(q):
            k0 = sum(XSPLIT[:q])
            nk = XSPLIT[q]
            xt = xp.tile([128, nk * C], DT, tag=f"x{q}")
            nc.sync.dma_start(xt[:], x_ds[q].ap())
            x_t.append((xt, k0, nk))

        wgu = {}
        def _load_w13_half(m, which):
            # one g (which=0) or u (which=1) row-block of w13, 512KB
            t = wp.tile([128, GW], DT, tag="w13")
            nc.sync.dma_start(t[:], w13_d.ap()[2 * m + which])
            wgu[(m, which)] = t

        # Issue order = consumption order of the restructured ramp below:
        # m0 k0-7 halves, then all of m1, then m0 k8-15, then m2...
        _load_x(0)       # k0: first matmul dep (tiny, lands first)
        _load_w0(0, 0)   # wg k0-7
        _load_x(1)       # k1-3
        _load_x(2)       # k4-7
        _load_w0(1, 0)   # wu k0-7
        _load_w13_half(1, 0)
        _load_x(3)       # k8-11
        _load_x(4)       # k12-15 (before wu1: m1's g-chain consumes it first)
        _load_w13_half(1, 1)
        _load_w0(0, 1)   # wg k8-15
        _load_w0(1, 1)   # wu k8-15
        _load_w13_half(2, 0)
        _load_w13_half(2, 1)

        def xsl(k):
            for xt, k0, nk in x_t:
                if k0 <= k < k0 + nk:
                    return xt[:, (k - k0) * C: (k - k0 + 1) * C]
            raise AssertionError(k)

        # --- stage 1 + 2 ---------------------------------------------------
        # Ramp order: m0 k0-7 (paced by the x chunk supply), then ALL of
        # m1 (its weights land early), then m0 k8-15 (deps long-arrived by
        # now, so no per-chunk completion-semaphore stalls), then m2..m10.
        act_t = [None] * IT

        def _weights(m):
            for which in (0, 1):
                if (m, which) not in wgu:
                    _load_w13_half(m, which)
            g_t = wgu.pop((m, 0))
            u_t = wgu.pop((m, 1))
            return ([g_t[:, k * 128: (k + 1) * 128] for k in range(HT)],
                    [u_t[:, k * 128: (k + 1) * 128] for k in range(HT)])

        def _chains(g_w, u_w, g_ps, u_ps, ks, start, stop):
            for k in ks:
                nc.tensor.matmul(g_ps[:], g_w[k], xsl(k),
                                 start=(start and k == ks[0]),
                                 stop=(stop and k == ks[-1]))
            for k in ks:
                nc.tensor.matmul(u_ps[:], u_w[k], xsl(k),
                                 start=(start and k == ks[0]),
                                 stop=(stop and k == ks[-1]))

        def _finish(m, g_ps, u_ps):
            sg = sp.tile([128, C], F32, tag="sg")
            nc.scalar.activation(sg[:], g_ps[:], AF.Silu)
            at = ap.tile([128, C], DT, tag=f"act{m}")
            nc.vector.tensor_mul(at[:], sg[:], u_ps[:])
            act_t[m] = at

        g_w0 = [w0[(0, k // 8)][:, (k % 8) * 128: (k % 8 + 1) * 128]
                for k in range(HT)]
        u_w0 = [w0[(1, k // 8)][:, (k % 8) * 128: (k % 8 + 1) * 128]
                for k in range(HT)]
        g_ps0 = psg.tile([128, C], F32, tag="ps")
        u_ps0 = psg.tile([128, C], F32, tag="ps")
        # filler warmups inside the DMA-paced ramp keep PE activity dense
        # through the x-chunk waits, so the HAM clock-gate stays at 8/8
        for k in range(HT // 2):
            nc.tensor.matmul(g_ps0[:], g_w0[k], xsl(k),
                             start=(k == 0), stop=False)
            if k in (0, 3, 5):
                nc.tensor.matmul(warm_ps[:], zw[:], zx[:], start=True, stop=True)
                nc.tensor.matmul(warm_ps[:], zw[:], zx[:], start=True, stop=True)
        for k in range(HT // 2):
            nc.tensor.matmul(u_ps0[:], u_w0[k], xsl(k),
                             start=(k == 0), stop=False)

        g_w1, u_w1 = _weights(1)
        g_ps1 = psg.tile([128, C], F32, tag="ps")
        u_ps1 = psg.tile([128, C], F32, tag="ps")
        _chains(g_w1, u_w1, g_ps1, u_ps1, list(range(HT)),
                start=True, stop=True)
        _finish(1, g_ps1, u_ps1)

        _chains(g_w0, u_w0, g_ps0, u_ps0, list(range(HT // 2, HT)),
                start=False, stop=True)
        _finish(0, g_ps0, u_ps0)

        for m in range(2, IT):
            g_w, u_w = _weights(m)
            g_ps = psg.tile([128, C], F32, tag="ps")
            u_ps = psg.tile([128, C], F32, tag="ps")
            _chains(g_w, u_w, g_ps, u_ps, list(range(HT)),
                    start=True, stop=True)
            _finish(m, g_ps, u_ps)

        # --- stage 3 -------------------------------------------------------
        # w2 pair p is DMA'd two blocks ahead of first use (pairs 0,1 queue
        # behind the last w13 transfers; later pairs issue inside the loop).
        w2t = {}
        def _load_w2(p):
            t = w2p.tile([128, 2 * IT * 128], DT, tag=f"w2_{p % 2}")
            nc.sync.dma_start(t[:], w2_d.ap()[p])
            w2t[p] = t

        _load_w2(0)
        _load_w2(1)
        for m in range(HT):
            p = m // 2
            if m % 2 == 0 and p + 2 <= HT // 2 - 1:
                _load_w2(p + 2)
            wt = w2t[p]
            base = (m % 2) * IT * 128
            y_ps = psy.tile([128, C], F32, tag="y")
            for k in range(IT):
                nc.tensor.matmul(
                    y_ps[:], wt[:, base + k * 128: base + (k + 1) * 128],
                    act_t[k][:], start=(k == 0), stop=(k == IT - 1),
                )
            if m % 2 == 1:
                del w2t[p]
            y_sb = yp.tile([128, C], DT, tag="yout")
            nc.vector.tensor_copy(y_sb[:], y_ps[:])
            # scalar HWDGE ring: keeps y stores off the sync ring's queue
            nc.scalar.dma_start(y_d.ap()[m], y_sb[:])

    nc.compile()
    return nc


def _get_nc(C):
    if C not in _cache:
        _cache[C] = _build_nc(C)
    return _cache[C]


def _prep_weights(w13, w2):
    """Pre-tile weights into the SBUF layouts the kernel DMAs verbatim."""
    GW = HT * 128
    wb = (
        w13.reshape(E, BT, 128, HT, 128)
        .transpose(0, 1, 4, 3, 2)
        .astype(NP_DT)
        .reshape(E, BT, 128, GW)
    )
    # halves interleaved: row 2m = g block m, row 2m+1 = u block m
    w13h = np.stack([wb[:, :IT], wb[:, IT:]], axis=2).reshape(E, 2 * IT, 128, GW)
    # block-0 quarters in first-use order: g k0-7, u k0-7, g k8-15, u k8-15
    w0q = np.stack(
        [wb[:, 0, :, : GW // 2], wb[:, IT, :, : GW // 2],
         wb[:, 0, :, GW // 2:], wb[:, IT, :, GW // 2:]],
        axis=1,
    )
    w2b = (
        w2.reshape(E, HT, 128, IT, 128)
        .transpose(0, 1, 4, 3, 2)
        .astype(NP_DT)
        .reshape(E, HT, 128, IT * 128)
    )
    w2_sb = np.concatenate([w2b[:, 0::2], w2b[:, 1::2]], axis=3)
    return (np.ascontiguousarray(w0q), np.ascontiguousarray(w13h),
            np.ascontiguousarray(w2_sb))


def _prep_x(xe, C):
    """[C, H] fp32 -> per-chunk arrays [128, nk*C] fp16, k-tiles side by side."""
    xt = xe.T.reshape(HT, 128, C).transpose(1, 0, 2)
    flat = np.ascontiguousarray(xt).reshape(128, HT * C).astype(NP_DT)
    out = {}
    k0 = 0
    for q, nk in enumerate(XSPLIT):
        out[f"x{q}"] = np.ascontiguousarray(flat[:, k0 * C: (k0 + nk) * C])
        k0 += nk
    return out


def kernel(
    hidden_states,
    topk_weights,
    topk_ids,
    w13,
    w2,
    num_global_tokens=None,
    max_num_tokens_per_gpu=None,
):
    from concourse.bass_utils import run_bass_kernel_spmd

    hs = np.asarray(hidden_states, dtype=np.float32)
    tw = np.asarray(topk_weights, dtype=np.float32)
    ti = np.asarray(topk_ids)
    w13 = np.asarray(w13, dtype=np.float32)
    w2 = np.asarray(w2, dtype=np.float32)

    assert hs.shape == (T, H), hs.shape
    assert w13.shape == (E, 2 * I, H), w13.shape
    assert w2.shape == (E, H, I), w2.shape

    # per-(token, expert) combine weights: sum of topk weights routed to e
    # (out-of-range ids contribute nothing, matching jax.nn.one_hot)
    comb = np.zeros((T, E), dtype=np.float32)
    for k in range(ti.shape[1]):
        col = ti[:, k]
        ok = (col >= 0) & (col < E)
        np.add.at(comb, (np.arange(T)[ok], col[ok]), tw[ok, k])

    idxs = [np.nonzero(comb[:, e])[0] for e in range(E)]
    need = max(len(ix) for ix in idxs)
    # token capacity: matmul N dim, multiple of 4 (x chunk layout)
    C = min(CMAX, max(64, -(-need // 4) * 4))
    nchunks = max(1, -(-need // C))

    w0q, w13h, w2_sb = _prep_weights(w13, w2)
    nc = _get_nc(C)

    trace = bool(os.environ.get("KERNEL_PROFILE"))
    out = np.zeros((T, H), dtype=np.float32)
    for chunk in range(nchunks):
        in_maps = []
        sels = []
        for e in range(E):
            sel = idxs[e][chunk * C: (chunk + 1) * C]
            xe = np.zeros((C, H), dtype=np.float32)
            xe[: len(sel)] = hs[sel]
            im = {"w0q": w0q[e], "w13h": w13h[e], "w2_sb": w2_sb[e]}
            im.update(_prep_x(xe, C))
            in_maps.append(im)
            sels.append(sel)
        if trace:
            try:
                res = run_bass_kernel_spmd(nc, in_maps, list(range(E)), trace=True)
                if res.exec_time_ns is not None:
                    print(f"HW exec time: {res.exec_time_ns} ns")
            except Exception:
                res = run_bass_kernel_spmd(nc, in_maps, list(range(E)))
        else:
            res = run_bass_kernel_spmd(nc, in_maps, list(range(E)))
        for e in range(E):
            sel = sels[e]
            if len(sel) == 0:
                continue
            y_sb = np.asarray(res.results[e]["y_sb"], dtype=np.float32)
            ye = y_sb.reshape(H, C).T  # [C, H]
            out[sel] += comb[sel, e][:, None] * ye[: len(sel)]
    return out
